# revision 1
# baseline (speedup 1.0000x reference)
"""Cross-attention block on 8 Trainium2 NeuronCores.

Computes, per batch b:
    xn = LN(x); cn = LN(cond)
    q = xn @ Wq; k = cn @ Wk; v = cn @ Wv   (8 heads x 64)
    out = softmax(q k^T / sqrt(64)) v
    y  = LN(out @ Wo + bo + x)

Sharding: 8 cores = 4 batches x 2 query-row halves (data parallel over
(batch, query-block)).  Each core recomputes LN(cond)/K/V for its batch
(duplicated across the 2 cores of a batch) and produces a disjoint
[1024, 512] slice of the output, so no collectives are needed.

On-core layout: activations are kept transposed (features on SBUF
partitions, tokens on the free axis).  The transposes are done by the
DMA XBAR on a bf16 DRAM bounce (LN body -> DRAM -> transposing read),
so the tensor engine only runs matmuls.  LN affine params are applied
post-transpose as per-partition scalars.  Attention scores are built
directly in S^T form (keys on partitions), which feeds O^T = V^T P^T
without transposing the 16.8M-element probability matrix.  Softmax runs
without max-subtraction (scores are ~N(0,1), |s| < ~7, exp is safe) and
the denominator is fused into the PV matmul as a 65th stationary column
of ones, so no separate denominator pass is needed.  Matmul inputs are
bf16, accumulation fp32.
"""

import functools

import numpy as np

B, N, M = 4, 2048, 2048
DQ, DC = 512, 768
H, DH = 8, 64
INNER = H * DH  # 512
P = 128
NQ = N // 2  # query rows per core
EPS = 1e-5
N_CORES = 8

FC_X = DQ // P  # 4 feature chunks of x
FC_C = DC // P  # 6 feature chunks of cond
IC = INNER // P  # 4 inner chunks
TQ = NQ // P  # 8 query-token chunks per core
TK = M // P  # 16 key-token chunks
NT = NQ // 512  # 2 query column tiles (transposed layout)
KNT = M // 512  # 4 key column tiles


def _emit(tc, io):
    import contextlib
    import math

    import concourse.bass as bass
    import concourse.mybir as mybir

    nc = tc.nc
    f32 = mybir.dt.float32
    bf16 = mybir.dt.bfloat16
    AF = mybir.ActivationFunctionType
    OP = mybir.AluOpType

    ctx = contextlib.ExitStack()
    with ctx:
        singles = ctx.enter_context(tc.tile_pool(name="singles", bufs=1))
        wstage = ctx.enter_context(tc.tile_pool(name="wstage", bufs=1))
        work = ctx.enter_context(tc.tile_pool(name="work", bufs=3))
        stat = ctx.enter_context(tc.tile_pool(name="stat", bufs=4))
        cenp = ctx.enter_context(tc.tile_pool(name="cenp", bufs=5))
        ppool = ctx.enter_context(tc.tile_pool(name="ppool", bufs=7))
        dram = ctx.enter_context(tc.tile_pool(name="dram", bufs=1, space="DRAM"))
        ps = ctx.enter_context(tc.tile_pool(name="ps", bufs=2, space="PSUM"))

        # ---- constants -------------------------------------------------
        from concourse.masks import make_identity

        ident = singles.tile([P, P], bf16, name="ident")
        make_identity(nc, ident)
        eps_t = singles.tile([P, 1], f32, name="eps_t")
        nc.vector.memset(eps_t, EPS)

        def bcast_load(vec_ap, width, name):
            """[width] dram vector -> [128, width] sbuf tile (same row on
            every partition)."""
            t = singles.tile([P, width], f32, name=name)
            bc = bass.AP(
                tensor=vec_ap.tensor,
                offset=vec_ap.offset,
                ap=[[0, P]] + [list(a) for a in vec_ap.ap],
            )
            nc.gpsimd.dma_start(out=t, in_=bc)
            return t

        def strip_load(vec_ap, chunks, name):
            """[chunks*128] dram vector -> [128, chunks] sbuf (feature-on-
            partition layout)."""
            t = singles.tile([P, chunks], f32, name=name)
            nc.sync.dma_start(out=t, in_=vec_ap.rearrange("(c p) -> p c", p=P))
            return t

        gx = strip_load(io["lnx_g"], FC_X, "gx")
        bx = strip_load(io["lnx_b"], FC_X, "bx")
        gc = strip_load(io["lnc_g"], FC_C, "gc")
        bc_ = strip_load(io["lnc_b"], FC_C, "bc")
        gf_bc = bcast_load(io["lnf_g"], DQ, "gf_bc")
        bf_bc = bcast_load(io["lnf_b"], DQ, "bf_bc")
        bo_bc = bcast_load(io["bo"], DQ, "bo_bc")

        # ---- weights: fp32 HBM -> bf16 SBUF, contraction on partitions --
        def load_weight(w_ap, din, name):
            kc = din // P
            stage = wstage.tile([P, kc, INNER], f32, tag="wstage", name=f"{name}_st")
            nc.sync.dma_start(
                out=stage, in_=w_ap.rearrange("(ko p) i -> p ko i", p=P)
            )
            wb = singles.tile([P, kc, INNER], bf16, name=name)
            nc.scalar.copy(out=wb, in_=stage)
            return wb

        wq_b = load_weight(io["Wq"], DQ, "wq_b")
        wk_b = load_weight(io["Wk"], DC, "wk_b")
        wv_b = load_weight(io["Wv"], DC, "wv_b")
        # Wo in head-major rows to match the 64-partition O^T layout.
        wo_st = wstage.tile([DH, H, DQ], f32, tag="wostage", name="wo_st")
        nc.sync.dma_start(
            out=wo_st, in_=io["Wo"].rearrange("(h p) d -> p h d", p=DH)
        )
        wo_b = singles.tile([DH, H, DQ], bf16, name="wo_b")
        nc.scalar.copy(out=wo_b, in_=wo_st)

        # ---- persistent activations ------------------------------------
        xnT = singles.tile([P, FC_X, NQ], bf16, name="xnT")  # LN(x)^T
        cnT = singles.tile([P, FC_C, M], bf16, name="cnT")  # LN(cond)^T
        QT = singles.tile([P, IC, NQ], bf16, name="QT")  # (q*scale)^T
        KT = singles.tile([P, IC, M], bf16, name="KT")  # k^T
        # v in token layout, one ones-column per head for the fused
        # softmax denominator: V_sb[:, mc, h, 0:64] = v, [..., 64] = 1.
        V_sb = singles.tile([P, TK, H, DH + 1], bf16, name="V_sb")
        nc.vector.memset(V_sb, 1.0)
        # attn out^T, head-major on 64 partitions
        OT = singles.tile([DH, H, NQ], bf16, name="OT")

        # ---- phase 1/2: LN + PE transpose into feature-major layout ----
        def ln_transpose(src_ap, width, tchunks, g_strip, b_strip, dst):
            fmax = math.gcd(512, width)
            nsub = width // fmax
            fc_n = width // P
            src = src_ap.rearrange("(t p) d -> p t d", p=P)
            for tg in range(tchunks // 4):
                cents, mvs = [], []
                std = stat.tile([P, 4], f32, tag="std", name="std")
                xs = []
                for tl in range(4):
                    t = tg * 4 + tl
                    x_t = work.tile([P, width], f32, tag="xin", bufs=4, name="x_t")
                    nc.sync.dma_start(out=x_t, in_=src[:, t])
                    if nsub == 1:
                        stats = stat.tile(
                            [P, 6], f32, tag="bnstats", bufs=6, name="stats"
                        )
                        nc.vector.bn_stats(out=stats, in_=x_t)
                    else:
                        xr = x_t.rearrange("p (s f) -> p s f", f=fmax)
                        stats = stat.tile(
                            [P, nsub, 6], f32, tag="bnstats", bufs=6, name="stats"
                        )
                        for s in range(nsub):
                            nc.vector.bn_stats(out=stats[:, s], in_=xr[:, s])
                    mv = stat.tile([P, 2], f32, tag="bnaggr", bufs=6, name="mv")
                    nc.vector.bn_aggr(out=mv, in_=stats)
                    nc.scalar.activation(
                        out=std[:, tl : tl + 1],
                        in_=mv[:, 1:2],
                        func=AF.Sqrt,
                        bias=eps_t,
                        scale=1.0,
                    )
                    xs.append(x_t)
                    mvs.append(mv)
                rstd = stat.tile([P, 4], f32, tag="rstd", name="rstd")
                nc.vector.reciprocal(out=rstd, in_=std)
                nmr = stat.tile([P, 4], f32, tag="nmr", name="nmr")
                for tl in range(4):
                    # -mean * rstd, for the fused ACT apply below
                    nc.vector.scalar_tensor_tensor(
                        out=nmr[:, tl : tl + 1],
                        in0=mvs[tl][:, 0:1],
                        scalar=-1.0,
                        in1=rstd[:, tl : tl + 1],
                        op0=OP.mult,
                        op1=OP.mult,
                    )
                for tl in range(4):
                    cen = cenp.tile([P, width], bf16, tag="cen", name="cen")
                    nc.scalar.activation(
                        out=cen,
                        in_=xs[tl],
                        func=AF.Identity,
                        bias=nmr[:, tl : tl + 1],
                        scale=rstd[:, tl : tl + 1],
                    )
                    cents.append(cen)
                for fc in range(fc_n):
                    tp = ps.tile([P, 4, P], bf16, tag="st", bufs=3, name="tp")
                    for tl in range(4):
                        nc.tensor.transpose(
                            tp[:, tl], cents[tl][:, fc * P : (fc + 1) * P], ident
                        )
                    # dst = tp * g[fc] + b[fc]   (per-partition scalars)
                    nc.vector.tensor_scalar(
                        out=dst[:, fc, tg * 512 : (tg + 1) * 512],
                        in0=tp,
                        scalar1=g_strip[:, fc : fc + 1],
                        scalar2=b_strip[:, fc : fc + 1],
                        op0=OP.mult,
                        op1=OP.add,
                    )

        ln_transpose(io["x"], DQ, TQ, gx, bx, xnT)
        ln_transpose(io["cond"], DC, TK, gc, bc_, cnT)

        # ---- phase 3: projections --------------------------------------
        scale = float(DH) ** -0.5
        # QT = Wq^T xn^T (scaled); keep each Wq chunk stationary across nt.
        for m in range(IC):
            qps = [
                ps.tile([P, 512], f32, tag="acc", bufs=2, name=f"ps_q{nt}")
                for nt in range(NT)
            ]
            for k in range(FC_X):
                for nt in range(NT):
                    nc.tensor.matmul(
                        qps[nt],
                        lhsT=wq_b[:, k, m * P : (m + 1) * P],
                        rhs=xnT[:, k, nt * 512 : (nt + 1) * 512],
                        start=(k == 0),
                        stop=(k == FC_X - 1),
                    )
            for nt in range(NT):
                nc.vector.tensor_scalar(
                    out=QT[:, m, nt * 512 : (nt + 1) * 512],
                    in0=qps[nt],
                    scalar1=scale,
                    scalar2=None,
                    op0=OP.mult,
                )
        # KT = Wk^T cn^T; Wk chunk stationary across pairs of nt.
        for m in range(IC):
            for ng in range(KNT // 2):
                kps = [
                    ps.tile([P, 512], f32, tag="acc", bufs=2, name=f"ps_k{j}")
                    for j in range(2)
                ]
                for k in range(FC_C):
                    for j in range(2):
                        nt = ng * 2 + j
                        nc.tensor.matmul(
                            kps[j],
                            lhsT=wk_b[:, k, m * P : (m + 1) * P],
                            rhs=cnT[:, k, nt * 512 : (nt + 1) * 512],
                            start=(k == 0),
                            stop=(k == FC_C - 1),
                        )
                for j in range(2):
                    nt = ng * 2 + j
                    nc.vector.tensor_copy(
                        out=KT[:, m, nt * 512 : (nt + 1) * 512], in_=kps[j]
                    )
        # V = cn @ Wv, token layout, scattered per head next to the ones col.
        for mc in range(TK):
            ps_v = ps.tile([P, 512], f32, tag="acc", bufs=2, name="ps_v")
            for k in range(FC_C):
                nc.tensor.matmul(
                    ps_v,
                    lhsT=cnT[:, k, mc * P : (mc + 1) * P],
                    rhs=wv_b[:, k, :],
                    start=(k == 0),
                    stop=(k == FC_C - 1),
                )
            nc.vector.tensor_copy(
                out=V_sb[:, mc, :, 0:DH],
                in_=ps_v.rearrange("p (h d) -> p h d", h=H),
            )

        # ---- phase 4 + 5: attention, then Wo/LN per query tile ---------
        for nt in range(NT):
            for c in range(H // 2):
                hA, hB = 2 * c, 2 * c + 1
                q_a = QT[0:64, c, nt * 512 : (nt + 1) * 512]
                q_b = QT[64:128, c, nt * 512 : (nt + 1) * 512]
                ot = {
                    hA: ps.tile([P, 512], f32, tag="acc", bufs=2, name="ot_a"),
                    hB: ps.tile([P, 512], f32, tag="acc", bufs=2, name="ot_b"),
                }

                def emit_pv(mg, pp):
                    for j in range(2):
                        mc = mg * 2 + j
                        for h in (hA, hB):
                            nc.tensor.matmul(
                                ot[h][0 : DH + 1, :],
                                lhsT=V_sb[:, mc, h, :],
                                rhs=pp[h][:, j],
                                start=(mc == 0),
                                stop=(mc == TK - 1),
                            )

                pend = None
                for mg in range(TK // 2):
                    st_a = ps.tile([P, 2, 512], f32, tag="st", bufs=3, name="st_a")
                    st_b = ps.tile([P, 2, 512], f32, tag="st", bufs=3, name="st_b")
                    for j in range(2):
                        mc = mg * 2 + j
                        nc.tensor.matmul(
                            st_a[:, j],
                            lhsT=KT[0:64, c, mc * P : (mc + 1) * P],
                            rhs=q_a,
                            start=True,
                            stop=True,
                        )
                        nc.tensor.matmul(
                            st_b[:, j],
                            lhsT=KT[64:128, c, mc * P : (mc + 1) * P],
                            rhs=q_b,
                            start=True,
                            stop=True,
                        )
                    pa = ppool.tile([P, 2, 512], bf16, tag="p", name="pa")
                    pb = ppool.tile([P, 2, 512], bf16, tag="p", name="pb")
                    nc.scalar.activation(out=pa, in_=st_a, func=AF.Exp)
                    nc.scalar.activation(out=pb, in_=st_b, func=AF.Exp)
                    if pend is not None:
                        emit_pv(*pend)
                    pend = (mg, {hA: pa, hB: pb})
                emit_pv(*pend)

                # normalize: row DH of ot[h] holds the softmax denominator.
                # partition_broadcast only works from partition 0, so DMA the
                # reciprocal row from partition 64 down to a partition-0 tile.
                for h in (hA, hB):
                    rb = work.tile([P, 512], f32, tag="rb", bufs=2, name="rb")
                    nc.vector.reciprocal(
                        out=rb[DH : DH + 1, :], in_=ot[h][DH : DH + 1, :]
                    )
                    r0 = work.tile([1, 512], f32, tag="r0", bufs=2, name="r0")
                    nc.sync.dma_start(out=r0, in_=rb[DH : DH + 1, :])
                    nc.gpsimd.partition_broadcast(rb[0:DH, :], r0[0:1, :])
                    nc.vector.tensor_mul(
                        out=OT[:, h, nt * 512 : (nt + 1) * 512],
                        in0=ot[h][0:DH, :],
                        in1=rb[0:DH, :],
                    )

        # ---- phase 5: Wo projection + residual + final LN ---------------
        # Emitted after all attention so its ACT (sqrt) and DVE work queue
        # behind the exps instead of blocking them mid-stream.
        xr = io["x"].rearrange("(t p) d -> p t d", p=P)
        outr = io["out"].rearrange("(t p) d -> p t d", p=P)
        for t0 in (0, 4):
            ys, mvs = [], []
            stdf = stat.tile([P, 4], f32, tag="stdf", name="stdf")
            for tl in range(4):
                t = t0 + tl
                y_ps = ps.tile([P, 512], f32, tag="acc", bufs=2, name="y_ps")
                for h in range(H):
                    nc.tensor.matmul(
                        y_ps,
                        lhsT=OT[:, h, t * P : (t + 1) * P],
                        rhs=wo_b[:, h, :],
                        start=(h == 0),
                        stop=(h == H - 1),
                    )
                x_t = work.tile([P, DQ], f32, tag="xres", bufs=4, name="x_t2")
                nc.sync.dma_start(out=x_t, in_=xr[:, t])
                nc.gpsimd.tensor_add(out=x_t, in0=x_t, in1=bo_bc)
                y1 = work.tile([P, DQ], f32, tag="y1", bufs=5, name="y1")
                nc.vector.tensor_add(out=y1, in0=y_ps, in1=x_t)
                stats = stat.tile([P, 6], f32, tag="bnstats", bufs=6, name="statsf")
                nc.vector.bn_stats(out=stats, in_=y1)
                mv = stat.tile([P, 2], f32, tag="bnaggr", bufs=6, name="mvf")
                nc.vector.bn_aggr(out=mv, in_=stats)
                nc.scalar.activation(
                    out=stdf[:, tl : tl + 1],
                    in_=mv[:, 1:2],
                    func=AF.Sqrt,
                    bias=eps_t,
                    scale=1.0,
                )
                ys.append(y1)
                mvs.append(mv)
            rstdf = stat.tile([P, 4], f32, tag="rstdf", name="rstdf")
            nc.vector.reciprocal(out=rstdf, in_=stdf)
            for tl in range(4):
                t = t0 + tl
                y1 = ys[tl]
                nc.vector.tensor_scalar(
                    out=y1,
                    in0=y1,
                    scalar1=mvs[tl][:, 0:1],
                    scalar2=rstdf[:, tl : tl + 1],
                    op0=OP.subtract,
                    op1=OP.mult,
                )
                nc.vector.tensor_mul(out=y1, in0=y1, in1=gf_bc)
                nc.gpsimd.tensor_add(out=y1, in0=y1, in1=bf_bc)
                nc.sync.dma_start(out=outr[:, t], in_=y1)


@functools.cache
def _build_program():
    import concourse.bacc as bacc
    import concourse.mybir as mybir
    import concourse.tile as tile

    f32 = mybir.dt.float32
    nc = bacc.Bacc()
    io = {}
    io["x"] = nc.declare_dram_parameter("x", [NQ, DQ], f32, False)[:, :]
    io["cond"] = nc.declare_dram_parameter("cond", [M, DC], f32, False)[:, :]
    for name in ("lnx_g", "lnx_b"):
        io[name] = nc.declare_dram_parameter(name, [DQ], f32, False)[:]
    for name in ("lnc_g", "lnc_b"):
        io[name] = nc.declare_dram_parameter(name, [DC], f32, False)[:]
    io["Wq"] = nc.declare_dram_parameter("Wq", [DQ, INNER], f32, False)[:, :]
    io["Wk"] = nc.declare_dram_parameter("Wk", [DC, INNER], f32, False)[:, :]
    io["Wv"] = nc.declare_dram_parameter("Wv", [DC, INNER], f32, False)[:, :]
    io["Wo"] = nc.declare_dram_parameter("Wo", [INNER, DQ], f32, False)[:, :]
    for name in ("bo", "lnf_g", "lnf_b"):
        io[name] = nc.declare_dram_parameter(name, [DQ], f32, False)[:]
    io["out"] = nc.declare_dram_parameter("out", [NQ, DQ], f32, True)[:, :]

    with tile.TileContext(nc) as tc:
        _emit(tc, io)
    nc.compile()
    return nc


def _core_input_map(inputs, core):
    b, half = core // 2, core % 2
    m = {
        "x": np.ascontiguousarray(inputs["x"][b, half * NQ : (half + 1) * NQ]),
        "cond": np.ascontiguousarray(inputs["cond"][b]),
    }
    for name in (
        "lnx_g",
        "lnx_b",
        "lnc_g",
        "lnc_b",
        "Wq",
        "Wk",
        "Wv",
        "Wo",
        "bo",
        "lnf_g",
        "lnf_b",
    ):
        m[name] = np.asarray(inputs[name], dtype=np.float32)
    return m


TRACE = False
LAST_RESULTS = None


def kernel(**inputs):
    from concourse.bass_utils import run_bass_kernel_spmd

    global LAST_RESULTS
    nc = _build_program()
    in_maps = [_core_input_map(inputs, core) for core in range(N_CORES)]
    res = run_bass_kernel_spmd(
        nc,
        in_maps,
        list(range(N_CORES)),
        trace=TRACE,
        trace_cores=[0] if TRACE else None,
    )
    LAST_RESULTS = res
    out = np.empty((B, N, DQ), np.float32)
    for core in range(N_CORES):
        b, half = core // 2, core % 2
        out[b, half * NQ : (half + 1) * NQ] = res.results[core]["out"]
    return out

